# revision 18
# baseline (speedup 1.0000x reference)
"""Trainium2 Bass kernel for the NCE cosine-similarity loss.

Problem: x [65536, 1024] f32 viewed as 1024 batches x 64 rows (1 orig, 8 pos,
55 neg). Per batch: cos(orig,pos_i) and cos(pos_i,neg_j), logits/0.1,
loss = logsumexp([cp, cn_*]) - cp, mean over all (batch, pos).

Strategy (8 NeuronCores, data-parallel over batches, 128 batches/core):
 - Two batches share one 128-partition tile ("pair"): rows on partitions.
 - SWDGE cast-load fp32->bf16 (~410 GB/s measured), PE-transpose to
   [d-chunk, row] layout, full 128x128 Gram per pair via 8 accumulating
   bf16 matmuls.
 - Norms: one fused DVE tensor_tensor_reduce (G * I, row-sum) pulls the
   Gram diagonal straight out of PSUM. inv = sqrt(10/n2) folds the 1/tau
   logit scale into both cosine normalizations.
 - Only the 16 pos *columns* of the Gram become logits: one DVE
   scalar_tensor_tensor (G[:,pos] * inv_row * inv_col) + one ScalarE Exp.
 - Denominators and e^{L0} come from a tiny PE matmul: exp_sb^T @ mask4
   (mask4 selects neg+orig rows per batch, and the orig row alone),
   accumulated across pairs into a [16, 256] PSUM tile. One Ln at the end;
   host sums log(den) - L0 over the valid slots of the 8 cores.
"""

import sys

if "/opt/trn_rl_repo" not in sys.path:
    sys.path.insert(0, "/opt/trn_rl_repo")

import numpy as np

N_CORES = 8
ROWS_PER_CORE = 8192          # 128 batches x 64 rows
D = 1024
N_GROUPS = 8                  # dma groups of 8 pairs per core
N_QUADS = 16                  # quads of 4 pairs per core
N_PAIRS = 64                  # 2 batches per pair

_CACHE = {}


def _build(repeat=1, loop_n=0, stage=8, dma_once=False):
    import concourse.bacc as bacc
    import concourse.mybir as mybir
    import concourse.tile as tile

    dt = mybir.dt
    AF = mybir.ActivationFunctionType
    ALU = mybir.AluOpType

    nc = bacc.Bacc("TRN2", target_bir_lowering=False, debug=False, num_devices=N_CORES)
    x = nc.dram_tensor("x", [ROWS_PER_CORE, D], dt.float32, kind="ExternalInput")
    identb_d = nc.dram_tensor("identb", [128, 128], dt.bfloat16, kind="ExternalInput")
    identb4_d = nc.dram_tensor("identb4", [128, 512], dt.bfloat16, kind="ExternalInput")
    sel_d = nc.dram_tensor("sel", [4, 512], dt.bfloat16, kind="ExternalInput")
    mask4_d = nc.dram_tensor("mask4", [128, 4], dt.bfloat16, kind="ExternalInput")
    out_d = nc.dram_tensor("out", [16, 256], dt.float32, kind="ExternalOutput")

    # x rows (g j p) d: group g, pair-in-group j, partition p
    xg = x.rearrange("(g j p) d -> g p j d", g=N_GROUPS, j=8, p=128)

    with tile.TileContext(nc) as tc:
        from contextlib import ExitStack, nullcontext

        with ExitStack() as ctx:
            cpool = ctx.enter_context(tc.tile_pool(name="consts", bufs=1))
            rowp = ctx.enter_context(tc.tile_pool(name="row", bufs=3))
            tp = ctx.enter_context(tc.tile_pool(name="tgrp", bufs=6))
            tpsp = ctx.enter_context(tc.tile_pool(name="tps", bufs=3, space="PSUM"))
            gramp = ctx.enter_context(tc.tile_pool(name="gram", bufs=2, space="PSUM"))
            itpsp = ctx.enter_context(tc.tile_pool(name="itps", bufs=1, space="PSUM"))
            bcpsp = ctx.enter_context(tc.tile_pool(name="bcps", bufs=1, space="PSUM"))
            spsp = ctx.enter_context(tc.tile_pool(name="sps", bufs=1, space="PSUM"))
            sb = ctx.enter_context(tc.tile_pool(name="sb", bufs=2))
            scrp = ctx.enter_context(tc.tile_pool(name="scr", bufs=2))
            stg = ctx.enter_context(tc.tile_pool(name="stg", bufs=1))

            identb = cpool.tile([128, 128], dt.bfloat16)
            nc.sync.dma_start(out=identb[:], in_=identb_d[:])
            identb4 = cpool.tile([128, 4, 128], dt.bfloat16)
            nc.sync.dma_start(out=identb4.rearrange("p a b -> p (a b)"),
                              in_=identb4_d[:])
            selc = cpool.tile([4, 512], dt.bfloat16)
            nc.sync.dma_start(out=selc[:], in_=sel_d[:])
            mask4 = cpool.tile([128, 4], dt.bfloat16)
            nc.sync.dma_start(out=mask4[:], in_=mask4_d[:])

            # persistent [16, 256] PSUM accumulator: per pair p, columns
            # 4p..4p+3 = [denA, denB, e0A, e0B] for the 16 pos columns
            s_psum = spsp.tile([16, 256], dt.float32, tag="s_psum")
            lnout = stg.tile([16, 256], dt.float32, tag="lnout")

            def phase_a(q, row_tiles):
                """loads, transposes, grams, diag extraction for quad q"""
                if q % 2 == 0 and (not dma_once or not row_tiles):
                    G = 0 if dma_once else q // 2
                    row = rowp.tile([128, 8, D], dt.bfloat16, tag="row")
                    nc.gpsimd.dma_start(out=row[:], in_=xg[G])
                    row_tiles[G] = row
                row = row_tiles[0 if dma_once else q // 2]

                n2q = sb.tile([128, 4], dt.float32, tag="n2q")
                grams = []
                t_list = []
                for jj in range(4):
                    j_in_g = (q % 2) * 4 + jj
                    g_abs = 4 * q + jj
                    tps = tpsp.tile([128, 8, 128], dt.bfloat16, tag="tps")
                    for c in range(8):
                        nc.tensor.transpose(
                            tps[:, c, :],
                            row[:, j_in_g, c * 128 : (c + 1) * 128],
                            identb[:],
                        )
                    t = tp.tile([128, 8, 128], dt.bfloat16, tag="t")
                    if g_abs % 2 == 0:
                        # int32 bitcast halves the element count (2x DVE copy)
                        nc.vector.tensor_copy(
                            t.rearrange("p a b -> p (a b)").bitcast(dt.int32),
                            tps.rearrange("p a b -> p (a b)").bitcast(dt.int32),
                        )
                    else:
                        nc.scalar.copy(t.rearrange("p a b -> p (a b)"),
                                       tps.rearrange("p a b -> p (a b)"))
                    t_list.append(t)
                if stage < 2:
                    return n2q, grams
                # 4 grams of the quad packed into one PSUM bank
                gq = gramp.tile([128, 4, 128], dt.float32, tag="gram")
                for jj in range(4):
                    t = t_list[jj]
                    for c in range(8):
                        nc.tensor.matmul(
                            gq[:, jj, :],
                            t[:, c, :],
                            t[:, c, :],
                            start=(c == 0),
                            stop=(c == 7),
                        )
                    grams.append(gq[:, jj, :])
                # diag(G) -> n2 for all 4 grams at once; reading the whole
                # gq tile also orders this after every gram write (the DVE
                # may not read a PSUM bank the PE is still writing)
                scr = scrp.tile([128, 4, 128], dt.bfloat16, tag="scr")
                nc.vector.tensor_mul(scr[:], gq[:], identb4[:])
                nc.vector.reduce_sum(n2q[:], scr[:], axis=mybir.AxisListType.X)
                return n2q, grams

            def phase_b(q, n2q, grams):
                """normalize pos columns, exp, PE-accumulated masked sums"""
                if stage < 4:
                    return
                rcpq = sb.tile([128, 4], dt.float32, tag="rcpq")
                nc.vector.reciprocal(rcpq[:], n2q[:])
                invq = sb.tile([128, 4], dt.float32, tag="invq")
                # inv = sqrt(10/n2): folds tau and both norm factors
                nc.scalar.activation(invq[:], rcpq[:], AF.Sqrt, scale=10.0)

                if stage < 5:
                    return
                invb = sb.tile([128, 4], dt.bfloat16, tag="invb")
                nc.vector.tensor_copy(invb[:], invq[:])
                itps = itpsp.tile([4, 128], dt.float32, tag="itps")
                nc.tensor.matmul(itps[:], invb[:], identb[:])
                itsb = sb.tile([4, 128], dt.bfloat16, tag="itsb")
                nc.vector.tensor_copy(itsb[:], itps[:])

                if stage < 6:
                    return
                # broadcast the 16 pos-col inv values across partitions,
                # all 4 pairs into one PSUM tile, then one copy to SBUF
                bcq = bcpsp.tile([128, 4, 2, 8], dt.float32, tag="bcps")
                for jj in range(4):
                    nc.tensor.matmul(
                        bcq[:, jj, :, :],
                        selc[:, jj * 128 : (jj + 1) * 128],
                        itsb.rearrange("p (h x) -> p h x", h=2)[:, :, 1:9],
                    )
                bcsb = sb.tile([128, 4, 2, 8], dt.bfloat16, tag="bcsb")
                nc.vector.tensor_copy(
                    bcsb.rearrange("p a h x -> p (a h x)"),
                    bcq.rearrange("p a h x -> p (a h x)"),
                )
                if stage < 7:
                    return
                # logits for the 16 pos columns: G * inv_row * inv_col
                logits = sb.tile([128, 4, 2, 8], dt.float32, tag="logits")
                for jj in range(4):
                    nc.vector.scalar_tensor_tensor(
                        logits[:, jj, :, :],
                        grams[jj].rearrange("p (h x) -> p h x", h=2)[:, :, 1:9],
                        invq[:, jj : jj + 1],
                        bcsb[:, jj, :, :],
                        op0=ALU.mult,
                        op1=ALU.mult,
                    )
                exp_sb = sb.tile([128, 4, 16], dt.bfloat16, tag="exp_sb")
                nc.scalar.activation(
                    exp_sb.rearrange("p a x -> p (a x)"),
                    logits.rearrange("p a h x -> p (a h x)"),
                    AF.Exp,
                )
                if stage < 8:
                    return
                for jj in range(4):
                    pair_abs = 4 * q + jj
                    # denominators + e^{L0}: exp_sb^T @ mask4 -> [16, 4]
                    nc.tensor.matmul(
                        s_psum[:, 4 * pair_abs : 4 * pair_abs + 4],
                        exp_sb[:, jj, :],
                        mask4[:],
                    )

            loop_cm = tc.For_i(0, loop_n, 1) if loop_n else nullcontext()
            with loop_cm:
                row_tiles = {}
                pending = None
                for q in range(N_QUADS * repeat):
                    q = q % N_QUADS
                    state = phase_a(q, row_tiles)
                    if pending is not None:
                        phase_b(pending[0], pending[1], pending[2])
                    pending = (q, *state)
                if pending is not None:
                    phase_b(pending[0], pending[1], pending[2])

                if stage >= 8:
                    nc.vector.tensor_copy(lnout[:], s_psum[:])
                else:
                    nc.vector.memset(lnout[:], 0.0)
            nc.gpsimd.dma_start(out=out_d[:], in_=lnout[:])

    nc.compile()
    return nc


def _consts():
    import ml_dtypes

    bf = ml_dtypes.bfloat16
    identb = np.eye(128, dtype=bf)
    identb4 = np.tile(np.eye(128, dtype=bf), (1, 4))
    # sel: one-hot row selector, sel[k, j*128 + p] = (k == j)
    sel = np.zeros((4, 512), dtype=bf)
    for jj in range(4):
        sel[jj, jj * 128 : (jj + 1) * 128] = 1.0
    # mask4 columns: [denominator rows A, denominator rows B, orig A, orig B]
    # denominator rows = orig + negs (rows 0, 9..63 of each 64-row batch)
    mask4 = np.zeros((128, 4), dtype=bf)
    mask4[0, 0] = 1.0
    mask4[9:64, 0] = 1.0
    mask4[64, 1] = 1.0
    mask4[73:128, 1] = 1.0
    mask4[0, 2] = 1.0
    mask4[64, 3] = 1.0
    return identb, identb4, sel, mask4


def kernel(x, labels=None, **_unused):
    from concourse.bass_utils import run_bass_kernel_spmd

    x = np.ascontiguousarray(np.asarray(x, dtype=np.float32))
    assert x.shape == (N_CORES * ROWS_PER_CORE, D), x.shape

    if "nc" not in _CACHE:
        _CACHE["nc"] = _build()
    nc = _CACHE["nc"]

    identb, identb4, sel, mask4 = _consts()
    in_maps = [
        {
            "x": x[i * ROWS_PER_CORE : (i + 1) * ROWS_PER_CORE],
            "identb": identb,
            "identb4": identb4,
            "sel": sel,
            "mask4": mask4,
        }
        for i in range(N_CORES)
    ]
    res = run_bass_kernel_spmd(nc, in_maps, list(range(N_CORES)))

    total = 0.0
    for r in res.results:
        o = np.log(r["out"].astype(np.float64)).reshape(16, 64, 4)
        # loss term = log(denominator) - L0, valid slots:
        #   pos cols 0..7  (batch A): den col 0, ln(e^{L0}) col 2
        #   pos cols 8..15 (batch B): den col 1, ln(e^{L0}) col 3
        total += o[0:8, :, 0].sum() - o[0:8, :, 2].sum()
        total += o[8:16, :, 1].sum() - o[8:16, :, 3].sum()
    loss = total / (1024 * 8)
    return np.array(loss, dtype=np.float32)
